# revision 29
# baseline (speedup 1.0000x reference)
"""Self-contained Trainium2 (Bass/Tile) kernel for the causal-attention module.

Problem shapes (hardcoded): x [2, 2048, 2048] fp32, rotary_emb [2048, 64] fp32,
gamma [2048] fp32, Wq [2048, 2048], Wkv [2048, 4096], Wout [2048, 2048] fp32.

Sharding: 8 NeuronCores = 2 batches (data parallel) x 4 head groups of 8
heads (tensor parallel).  Each core computes a full [2048, 2048] partial
output in bf16 (its head group's contribution through Wout's row block); the
host sums the 4 partials per batch in fp32.  Measured: 587.8us HW exec,
rel err 4.3e-3 (baseline: 751.1us, 3.5e-3).

Host prep: RMSNorm (gamma folded) is applied on the host and the normalized
activations are shipped pre-transposed as xn^T [dim, tok] bf16 — no on-chip
transpose pass, no sum-of-squares/rsqrt chain, half the x DMA.

Per-core kernel (matmuls bf16, fp32 PSUM):
  - K^T d-major [2 heads x 64d, tok] per head-pair, weights preloaded
    (chunked DMA so the first projections start as soon as the first
    activation tiles land).  rotate_half's partition permutation runs on
    the PE via a host-sent permutation matrix (no swap DMAs).  V natural
    [tok, h, 64] + ones column (softmax denominator free in the AV matmul).
  - Attention per 512-token i-block: scores S^T[j,i] (contraction 64), exp
    on ScalarE (no running max needed), causal via column clipping + a
    triangular mask multiply on diagonal blocks, AV accumulated in PSUM.
  - Software pipelining: Q projection+rotary for i-block i+1, V projection
    for i-blocks 4(i+1)..4(i+2), and the Wout projection of i-block i-2 are
    interleaved between the scores and AV matmuls as filler so TensorE never
    waits on ScalarE's exp.  Out-projection is deliberately scheduled two
    i-blocks late (heavier fill for the late, exp-heavy i-blocks).
  - The AV PSUM tile is cast to SBUF bf16 immediately and the denominator
    rows staged out (frees the PSUM bank for the next head pair in ~2us);
    1/d runs as a DVE reciprocal off the critical path (ScalarE reciprocal
    is blocked and GpSimd/custom-DVE alternatives miscompute on HW) and is
    broadcast across partitions with a rank-1 bf16 matmul.
"""

from contextlib import ExitStack

import numpy as np
import ml_dtypes

B, N, DIM = 2, 2048, 2048
HEADS_TOTAL, DH = 32, 64
N_CORES = 8
GROUPS = 4
HEADS = HEADS_TOTAL // GROUPS      # heads per core
HD = HEADS * DH                    # 512
IB = 512                           # query i-block width

_CACHED = {}


def _build():
    import concourse.tile as tile
    from concourse import mybir, bacc

    F32 = mybir.dt.float32
    BF16 = mybir.dt.bfloat16
    AF = mybir.ActivationFunctionType
    ALU = mybir.AluOpType

    n_ct = DIM // 128      # 16 contraction blocks
    n_tt = N // 128        # 16 token blocks
    n_ib = N // IB         # 4 i-blocks
    n_hb = HD // 128       # 4 head pairs
    jpi = IB // 128        # 4 j-blocks per i-block
    scale = DH ** -0.5

    nc = bacc.Bacc(None)
    xnT_d = nc.declare_dram_parameter("xnT", [DIM, N], BF16, isOutput=False)
    wq_d = nc.declare_dram_parameter("wq", [DIM, HD], BF16, isOutput=False)
    wk_d = nc.declare_dram_parameter("wk", [DIM, HD], BF16, isOutput=False)
    wv_d = nc.declare_dram_parameter("wv", [DIM, HD], BF16, isOutput=False)
    wout_d = nc.declare_dram_parameter("wout", [HD, DIM], BF16, isOutput=False)
    cosr_d = nc.declare_dram_parameter("cosr", [128, N], BF16, isOutput=False)
    sinr_d = nc.declare_dram_parameter("sinr", [128, N], BF16, isOutput=False)
    tri_d = nc.declare_dram_parameter("tri", [128, 128], BF16, isOutput=False)
    perm_d = nc.declare_dram_parameter("perm", [128, 128], BF16, isOutput=False)
    out_d = nc.declare_dram_parameter("out", [N, DIM], BF16, isOutput=True)

    ctx = ExitStack()
    with ctx:
        tc = ctx.enter_context(tile.TileContext(nc))
        pers = ctx.enter_context(tc.tile_pool(name="pers", bufs=1))
        wqp = ctx.enter_context(tc.tile_pool(name="wqp", bufs=2))
        qtp = ctx.enter_context(tc.tile_pool(name="qtp", bufs=2))
        epool = ctx.enter_context(tc.tile_pool(name="epool", bufs=3))
        rot = ctx.enter_context(tc.tile_pool(name="rot", bufs=2))
        ontp = ctx.enter_context(tc.tile_pool(name="ontp", bufs=3))
        osbp = ctx.enter_context(tc.tile_pool(name="osbp", bufs=2))
        ocp = ctx.enter_context(tc.tile_pool(name="ocp", bufs=2))
        bcp = ctx.enter_context(tc.tile_pool(name="bcp", bufs=1))
        ps = ctx.enter_context(tc.tile_pool(name="ps", bufs=2, space="PSUM"))
        ps_sc = ctx.enter_context(tc.tile_pool(name="pssc", bufs=1, space="PSUM"))
        ps_av = ctx.enter_context(tc.tile_pool(name="psav", bufs=1, space="PSUM"))

        dmae = [nc.sync, nc.gpsimd]

        xnT = [pers.tile([128, N], BF16, tag=f"xnT{c}", name=f"xnT{c}")
               for c in range(n_ct)]
        kt = [pers.tile([128, N], BF16, tag=f"kt{h}", name=f"kt{h}")
              for h in range(n_hb)]
        vst = [pers.tile([128, HEADS, DH + 1], BF16, tag=f"v{t}", name=f"v{t}")
               for t in range(n_tt)]
        wk_sb = pers.tile([128, n_ct, HD], BF16, tag="wk")
        wv_sb = pers.tile([128, n_ct, HD], BF16, tag="wv")
        wout_sb = pers.tile([128, n_hb, DIM], BF16, tag="wout")
        crep = pers.tile([128, N], BF16, tag="crep")
        srep = pers.tile([128, N], BF16, tag="srep")
        tri = pers.tile([128, 128], BF16, tag="tri")
        perm = pers.tile([128, 128], BF16, tag="perm")
        onesb = pers.tile([128, 128], BF16, tag="onesb")

        # startup: interleave wk chunks with xnT tiles so the K projection
        # pipeline starts as soon as the first tiles land; tables next
        # (needed by the first rotary), wv/wout last.
        wkr = wk_d.rearrange("(c p) h -> p c h", p=128)
        for ch in range(4):
            dmae[ch % 2].dma_start(wk_sb[:, ch * 4:(ch + 1) * 4, :],
                                   wkr[:, ch * 4:(ch + 1) * 4, :])
            if ch == 0:
                nc.sync.dma_start(crep[:], cosr_d[:])
                nc.gpsimd.dma_start(srep[:], sinr_d[:])
            for c in range(ch * 4, ch * 4 + 4):
                dmae[(c + 1) % 2].dma_start(out=xnT[c][:],
                                            in_=xnT_d[c * 128:(c + 1) * 128, :])
        nc.sync.dma_start(tri[:], tri_d[:])
        nc.sync.dma_start(perm[:], perm_d[:])
        nc.gpsimd.dma_start(wv_sb[:], wv_d.rearrange("(c p) h -> p c h", p=128))
        nc.sync.dma_start(wout_sb[:], wout_d.rearrange("(g p) e -> p g e", p=128))
        nc.vector.memset(onesb[:], 1.0)
        for t in range(n_tt):
            nc.vector.memset(vst[t][:, :, DH:DH + 1], 1.0)

        def rotary(ps_ap, dsl, tcl):
            """psum q/k [128, IB] -> dsl (bf16 slice), rotary applied.
            rotate_half's partition permutation runs on the PE (perm matmul);
            the sigma'd copy is consumed straight from PSUM."""
            raw = rot.tile([128, IB], BF16, tag="raw", name="raw")
            nc.vector.tensor_copy(raw[:], ps_ap[:])
            sq = ps.tile([128, IB], F32, tag="qps", name="sq")
            nc.tensor.matmul(sq[:], lhsT=perm[:], rhs=raw[:],
                             start=True, stop=True)
            tmp = rot.tile([128, IB], BF16, tag="tmp", name="tmp")
            nc.vector.tensor_mul(out=tmp[:], in0=raw[:], in1=crep[:, tcl])
            nc.vector.tensor_mul(out=dsl, in0=sq[:], in1=srep[:, tcl])
            nc.vector.tensor_add(out=dsl, in0=dsl, in1=tmp[:])

        def gen_kproj(hb, tc4):
            st = {}
            tcl = slice(tc4 * IB, (tc4 + 1) * IB)

            def mm(c0, c1):
                if c0 == 0:
                    st["ps"] = ps.tile([128, IB], F32, tag="qps", name="psk")
                for c in range(c0, c1):
                    nc.tensor.matmul(st["ps"][:],
                                     lhsT=wk_sb[:, c, hb * 128:(hb + 1) * 128],
                                     rhs=xnT[c][:, tcl],
                                     start=(c == 0), stop=(c == n_ct - 1))

            def fin():
                rotary(st["ps"], kt[hb][:, tcl], tcl)

            return [lambda: mm(0, 8), lambda: mm(8, 16), fin]

        def gen_qproj(ib, hb):
            st = {}
            tcl = slice(ib * IB, (ib + 1) * IB)

            def dma():
                slab = wqp.tile([128, n_ct, 128], BF16, tag="wq", name="wqs")
                dmae[(ib + hb) % 2].dma_start(
                    slab[:],
                    wq_d[:, hb * 128:(hb + 1) * 128].rearrange("(c p) m -> p c m",
                                                               p=128))
                st["slab"] = slab

            def mm(c0, c1):
                if c0 == 0:
                    st["ps"] = ps.tile([128, IB], F32, tag="qps", name="psq")
                for c in range(c0, c1):
                    nc.tensor.matmul(st["ps"][:], lhsT=st["slab"][:, c, :],
                                     rhs=xnT[c][:, tcl],
                                     start=(c == 0), stop=(c == n_ct - 1))

            def fin():
                qt_t = qtp.tile([128, IB], BF16, tag=f"qt{hb}", name=f"qt{hb}")
                qt_slot[(ib, hb)] = qt_t
                rotary(st["ps"], qt_t[:], tcl)

            return [dma, lambda: mm(0, 8), lambda: mm(8, 16), fin]

        def gen_vproj(t):
            st = {}

            def mm(c0, c1):
                if c0 == 0:
                    st["ps"] = ps.tile([128, IB], F32, tag="qps", name="psv")
                for c in range(c0, c1):
                    nc.tensor.matmul(st["ps"][:],
                                     lhsT=xnT[c][:, t * 128:(t + 1) * 128],
                                     rhs=wv_sb[:, c, :],
                                     start=(c == 0), stop=(c == n_ct - 1))

            def fin():
                nc.vector.tensor_copy(
                    vst[t][:, :, 0:DH],
                    st["ps"][:].rearrange("p (h d) -> p h d", h=HEADS))

            return [lambda: mm(0, 8), lambda: mm(8, 16), fin]

        qt_slot = {}
        ont_ap = {}

        def gen_outproj(i):
            """Wout projection of i-block i; bf16 [128, 2048] rows, 1 DMA per
            token block."""
            thunks = []
            for ic in range(4):
                st = {}
                for ec in range(4):
                    def th(ic=ic, ec=ec, st=st):
                        if ec == 0:
                            st["oc"] = ocp.tile([128, DIM], BF16, tag="oc",
                                                name="oc")
                        pso = ps.tile([128, 512], F32, tag="po", name="pso")
                        for hp in range(n_hb):
                            nc.tensor.matmul(
                                pso[:],
                                lhsT=ont_ap[(i, hp)][:, ic * 128:(ic + 1) * 128],
                                rhs=wout_sb[:, hp, ec * 512:(ec + 1) * 512],
                                start=(hp == 0), stop=(hp == n_hb - 1))
                        nc.scalar.copy(
                            st["oc"][:, ec * 512:(ec + 1) * 512], pso[:])
                        if ec == 3:
                            r0 = i * IB + ic * 128
                            dmae[(i + ic) % 2].dma_start(
                                out=out_d[r0:r0 + 128, :], in_=st["oc"][:])
                    thunks.append(th)
            return thunks

        def pipeline_units(units):
            """Flatten unit thunk-lists, delaying each unit's fin until after
            the next unit's first matmul thunk (so the fin's PE perm-matmul
            never head-of-line blocks the tensor queue on the DVE drain)."""
            seq = []
            pending = None
            for u in units:
                *body, fin = u
                seq.append(body[0])
                if pending is not None:
                    seq.append(pending)
                seq += body[1:]
                pending = fin
            if pending is not None:
                seq.append(pending)
            return seq

        # ---- phase 1: K (all), V (first 4 blocks), Q (i-block 0) ----
        units = [gen_kproj(hb, tc4) for hb in range(n_hb)
                 for tc4 in range(n_ib)]
        units += [gen_vproj(t) for t in range(jpi)]
        units += [gen_qproj(0, hb) for hb in range(n_hb)]
        for th in pipeline_units(units):
            th()

        # ---- phase 2: attention with pipelined filler ----
        pending = [None]
        for i in range(n_ib):
            funits = []
            if i < n_ib - 1:
                for hb in range(n_hb):
                    funits.append(gen_qproj(i + 1, hb))
                for t in range(jpi * (i + 1), jpi * (i + 2)):
                    funits.append(gen_vproj(t))
            fillers = pipeline_units(funits) if funits else []
            if i >= 2:
                fillers += gen_outproj(i - 2)
            if i == n_ib - 1:
                fillers += gen_outproj(i - 1)

            n_jb = jpi * (i + 1)
            total_steps = n_hb * n_jb
            done = 0
            step = 0
            for hp in range(n_hb):
                if pending[0] is not None and i == 0 and hp == 0:
                    pass  # flushed at jb==1 below
                o_ps = [ps_av.tile([DH + 1, IB], F32, tag=f"oav{k}",
                                   name=f"oav{k}") for k in (0, 1)]
                for jb in range(n_jb):
                    delta = jb - jpi * i
                    v0 = max(delta, 0) * 128
                    psl = slice(v0, IB)
                    jsl = slice(jb * 128, (jb + 1) * 128)
                    e_t = []
                    for k in (0, 1):
                        hsl = slice(k * 64, (k + 1) * 64)
                        s_ps = ps_sc.tile([128, IB], F32, tag=f"sc{k}",
                                          name=f"sc{k}")
                        nc.tensor.matmul(s_ps[:, psl], lhsT=kt[hp][hsl, jsl],
                                         rhs=qt_slot[(i, hp)][hsl, psl],
                                         start=True, stop=True)
                        e = epool.tile([128, IB], BF16, tag=f"e{k}",
                                       name=f"e{k}")
                        nc.scalar.activation(out=e[:, psl], in_=s_ps[:, psl],
                                             func=AF.Exp, scale=scale)
                        if delta >= 0:
                            dsl = slice(v0, v0 + 128)
                            nc.gpsimd.tensor_mul(out=e[:, dsl], in0=e[:, dsl],
                                                 in1=tri[:])
                        e_t.append(e)
                    if jb == 1 and pending[0] is not None:
                        pending[0]()
                        pending[0] = None
                    # filler between scores and AV: absorbs the exp latency
                    step += 1
                    want = len(fillers) * step // total_steps
                    while done < want:
                        fillers[done]()
                        done += 1
                    for k in (0, 1):
                        h = hp * 2 + k
                        nc.tensor.matmul(o_ps[k][:, psl],
                                         lhsT=vst[jb][:, h, :],
                                         rhs=e_t[k][:, psl],
                                         start=(jb == 0), stop=(jb == n_jb - 1))

                # AV psum -> one sbuf bf16 tile (k0 rows 0:64, k1 rows
                # 64:128 via the psum-source partition shift); 1/d via the
                # single-instruction DVE approx reciprocal straight from the
                # psum denominator rows; rank-1 f32r broadcast; normalize.
                osb = osbp.tile([128, IB], BF16, tag="osb", name="osb")
                nc.vector.tensor_copy(osb[0:DH, :], o_ps[0][0:DH, :])
                nc.vector.tensor_copy(osb[64:128, :], o_ps[1][0:DH, :])
                rec = bcp.tile([128, 2 * IB], F32, tag="rec", name="rec")
                rec_bf = bcp.tile([128, 2 * IB], BF16, tag="recb", name="rec_bf")
                nc.vector.tensor_copy(rec_bf[64:65, 0:IB],
                                      o_ps[0][DH:DH + 1, :])
                nc.vector.tensor_copy(rec_bf[64:65, IB:2 * IB],
                                      o_ps[1][DH:DH + 1, :])
                nc.vector.reciprocal(out=rec[64:65, 0:IB],
                                     in_=rec_bf[64:65, 0:IB])
                nc.vector.reciprocal(out=rec[64:65, IB:2 * IB],
                                     in_=rec_bf[64:65, IB:2 * IB])
                nc.vector.tensor_copy(rec_bf[64:65, :], rec[64:65, :])

                def norm(i=i, hp=hp, osb=osb, rec_bf=rec_bf):
                    # deferred into the next head-pair's j-loop so the pbc
                    # matmuls never head-of-line block on the DVE chain
                    ont_t = ontp.tile([128, IB], BF16, tag=f"ont{hp}",
                                      name=f"ont{hp}")
                    ont_ap[(i, hp)] = ont_t
                    for k in (0, 1):
                        pbc = ps.tile([128, IB], F32, tag="po", name="pbc")
                        nc.tensor.matmul(
                            pbc[:], lhsT=onesb[64:65, 0:128],
                            rhs=rec_bf[64:65, k * IB:(k + 1) * IB],
                            start=True, stop=True)
                        nc.vector.tensor_mul(
                            out=ont_t[k * 64:(k + 1) * 64, :],
                            in0=pbc[k * 64:(k + 1) * 64, :],
                            in1=osb[k * 64:(k + 1) * 64, :])
                pending[0] = norm

            while done < len(fillers):
                fillers[done]()
                done += 1

        if pending[0] is not None:
            pending[0]()
            pending[0] = None
        for th in gen_outproj(n_ib - 1):
            th()

    nc.compile()
    return nc


def get_nc():
    if "nc" not in _CACHED:
        _CACHED["nc"] = _build()
    return _CACHED["nc"]


def host_inputs(x, rotary_emb, gamma, Wq, Wkv, Wout):
    """Build the 8 per-core input dicts."""
    bf = ml_dtypes.bfloat16
    x = np.asarray(x, np.float32)
    g = np.asarray(gamma, np.float32)
    nrm = np.sqrt((x * x).sum(-1, keepdims=True))
    xn = x / np.maximum(nrm, 1e-12) * (DIM ** 0.5) * g
    Wq = np.asarray(Wq, np.float32)
    Wkv = np.asarray(Wkv, np.float32)
    Wk = Wkv[:, :HEADS_TOTAL * DH]
    Wv = Wkv[:, HEADS_TOTAL * DH:]
    Wout = np.asarray(Wout, np.float32)
    pos = np.asarray(rotary_emb, np.float32)
    cos = np.cos(pos).T
    sgn = np.concatenate([-np.ones(DH // 2), np.ones(DH // 2)]).astype(np.float32)
    sin = (np.sin(pos) * sgn[None, :]).T
    cosr = np.ascontiguousarray(np.tile(cos, (2, 1)).astype(bf))
    sinr = np.ascontiguousarray(np.tile(sin, (2, 1)).astype(bf))
    jj, ii = np.mgrid[0:128, 0:128]
    tri = np.ascontiguousarray((jj <= ii).astype(bf))
    # rotate_half partition permutation: out row p reads row sigma(p); sigma
    # swaps the 32-halves within each 64-row head slot.
    sigma = (np.arange(128) % 64 // 32 * -64 + 32) + np.arange(128)
    perm = np.zeros((128, 128), np.float32)
    perm[sigma, np.arange(128)] = 1.0
    perm = np.ascontiguousarray(perm.astype(bf))
    maps = []
    for core in range(N_CORES):
        b, gq = core // GROUPS, core % GROUPS
        hs = slice(gq * HD, (gq + 1) * HD)
        maps.append({
            "xnT": np.ascontiguousarray(xn[b].T.astype(bf)),
            "wq": np.ascontiguousarray(Wq[:, hs].astype(bf)),
            "wk": np.ascontiguousarray(Wk[:, hs].astype(bf)),
            "wv": np.ascontiguousarray(Wv[:, hs].astype(bf)),
            "wout": np.ascontiguousarray(Wout[hs, :].astype(bf)),
            "cosr": cosr, "sinr": sinr, "tri": tri, "perm": perm,
        })
    return maps


def run_cores(in_maps, trace=False, **kwargs):
    from concourse.bass_utils import run_bass_kernel_spmd
    nc = get_nc()
    return run_bass_kernel_spmd(nc, in_maps, list(range(N_CORES)), trace=trace,
                                **kwargs)


def kernel(x, rotary_emb, gamma, Wq, Wkv, Wout):
    in_maps = host_inputs(x, rotary_emb, gamma, Wq, Wkv, Wout)
    res = run_cores(in_maps, trace=False)
    out = np.zeros((B, N, DIM), np.float32)
    for core in range(N_CORES):
        b = core // GROUPS
        out[b] += res.results[core]["out"].astype(np.float32)
    return out


# revision 31
# speedup vs baseline: 1.2058x; 1.2058x over previous
"""Self-contained Trainium2 (Bass/Tile) kernel for the causal-attention module.

Problem shapes (hardcoded): x [2, 2048, 2048] fp32, rotary_emb [2048, 64] fp32,
gamma [2048] fp32, Wq [2048, 2048], Wkv [2048, 4096], Wout [2048, 2048] fp32.

Sharding: 8 NeuronCores = 2 batches (data parallel) x 4 head groups of 8
heads (tensor parallel).  Each core computes a full [2048, 2048] partial
output in bf16 (its head group's contribution through Wout's row block); the
host sums the 4 partials per batch in fp32.  Measured: 587.8us HW exec,
rel err 4.3e-3 (baseline: 751.1us, 3.5e-3).

Host prep: RMSNorm (gamma folded) is applied on the host and the normalized
activations are shipped pre-transposed as xn^T [dim, tok] bf16 — no on-chip
transpose pass, no sum-of-squares/rsqrt chain, half the x DMA.

Per-core kernel (matmuls bf16, fp32 PSUM):
  - K^T d-major [2 heads x 64d, tok] per head-pair, weights preloaded
    (chunked DMA so the first projections start as soon as the first
    activation tiles land).  rotate_half's partition permutation runs on
    the PE via a host-sent permutation matrix (no swap DMAs).  V natural
    [tok, h, 64] + ones column (softmax denominator free in the AV matmul).
  - Attention per 512-token i-block: scores S^T[j,i] (contraction 64), exp
    on ScalarE (no running max needed), causal via column clipping + a
    triangular mask multiply on diagonal blocks, AV accumulated in PSUM.
  - Software pipelining: Q projection+rotary for i-block i+1, V projection
    for i-blocks 4(i+1)..4(i+2), and the Wout projection of i-block i-2 are
    interleaved between the scores and AV matmuls as filler so TensorE never
    waits on ScalarE's exp.  Out-projection is deliberately scheduled two
    i-blocks late (heavier fill for the late, exp-heavy i-blocks).
  - The AV PSUM tile is cast to SBUF bf16 immediately and the denominator
    rows staged out (frees the PSUM bank for the next head pair in ~2us);
    1/d runs as a DVE reciprocal off the critical path (ScalarE reciprocal
    is blocked and GpSimd/custom-DVE alternatives miscompute on HW) and is
    broadcast across partitions with a rank-1 bf16 matmul.
"""

from contextlib import ExitStack

import numpy as np
import ml_dtypes

B, N, DIM = 2, 2048, 2048
HEADS_TOTAL, DH = 32, 64
N_CORES = 8
GROUPS = 4
HEADS = HEADS_TOTAL // GROUPS      # heads per core
HD = HEADS * DH                    # 512
IB = 512                           # query i-block width

_CACHED = {}


def _build():
    import concourse.tile as tile
    from concourse import mybir, bacc

    F32 = mybir.dt.float32
    BF16 = mybir.dt.bfloat16
    AF = mybir.ActivationFunctionType
    ALU = mybir.AluOpType

    n_ct = DIM // 128      # 16 contraction blocks
    n_tt = N // 128        # 16 token blocks
    n_ib = N // IB         # 4 i-blocks
    n_hb = HD // 128       # 4 head pairs
    jpi = IB // 128        # 4 j-blocks per i-block
    scale = DH ** -0.5

    nc = bacc.Bacc(None)
    xnT_d = nc.declare_dram_parameter("xnT", [DIM, N], BF16, isOutput=False)
    wq_d = nc.declare_dram_parameter("wq", [DIM, HD], BF16, isOutput=False)
    wk_d = nc.declare_dram_parameter("wk", [DIM, HD], BF16, isOutput=False)
    wv_d = nc.declare_dram_parameter("wv", [DIM, HD], BF16, isOutput=False)
    wout_d = nc.declare_dram_parameter("wout", [HD, DIM], BF16, isOutput=False)
    cosr_d = nc.declare_dram_parameter("cosr", [128, N], BF16, isOutput=False)
    sinr_d = nc.declare_dram_parameter("sinr", [128, N], BF16, isOutput=False)
    tri_d = nc.declare_dram_parameter("tri", [128, 128], BF16, isOutput=False)
    perm_d = nc.declare_dram_parameter("perm", [128, 128], BF16, isOutput=False)
    out_d = nc.declare_dram_parameter("out", [N, DIM], BF16, isOutput=True)

    ctx = ExitStack()
    with ctx:
        tc = ctx.enter_context(tile.TileContext(nc))
        pers = ctx.enter_context(tc.tile_pool(name="pers", bufs=1))
        wqp = ctx.enter_context(tc.tile_pool(name="wqp", bufs=2))
        qtp = ctx.enter_context(tc.tile_pool(name="qtp", bufs=2))
        epool = ctx.enter_context(tc.tile_pool(name="epool", bufs=3))
        rot = ctx.enter_context(tc.tile_pool(name="rot", bufs=2))
        ontp = ctx.enter_context(tc.tile_pool(name="ontp", bufs=3))
        osbp = ctx.enter_context(tc.tile_pool(name="osbp", bufs=2))
        ocp = ctx.enter_context(tc.tile_pool(name="ocp", bufs=2))
        bcp = ctx.enter_context(tc.tile_pool(name="bcp", bufs=1))
        ps = ctx.enter_context(tc.tile_pool(name="ps", bufs=2, space="PSUM"))
        ps_sc = ctx.enter_context(tc.tile_pool(name="pssc", bufs=1, space="PSUM"))
        ps_av = ctx.enter_context(tc.tile_pool(name="psav", bufs=1, space="PSUM"))

        dmae = [nc.sync, nc.gpsimd]

        xnT = [pers.tile([128, N], BF16, tag=f"xnT{c}", name=f"xnT{c}")
               for c in range(n_ct)]
        kt = [pers.tile([128, N], BF16, tag=f"kt{h}", name=f"kt{h}")
              for h in range(n_hb)]
        vst = [pers.tile([128, HEADS, DH + 1], BF16, tag=f"v{t}", name=f"v{t}")
               for t in range(n_tt)]
        wk_sb = pers.tile([128, n_ct, HD], BF16, tag="wk")
        wv_sb = pers.tile([128, n_ct, HD], BF16, tag="wv")
        wout_sb = pers.tile([128, n_hb, DIM], BF16, tag="wout")
        crep = pers.tile([128, N], BF16, tag="crep")
        srep = pers.tile([128, N], BF16, tag="srep")
        tri = pers.tile([128, 128], BF16, tag="tri")
        perm = pers.tile([128, 128], BF16, tag="perm")
        onesb = pers.tile([128, 128], BF16, tag="onesb")

        # startup: interleave wk chunks with xnT tiles so the K projection
        # pipeline starts as soon as the first tiles land; tables next
        # (needed by the first rotary), wv/wout last.
        wkr = wk_d.rearrange("(c p) h -> p c h", p=128)
        for ch in range(4):
            dmae[ch % 2].dma_start(wk_sb[:, ch * 4:(ch + 1) * 4, :],
                                   wkr[:, ch * 4:(ch + 1) * 4, :])
            if ch == 0:
                nc.sync.dma_start(crep[:], cosr_d[:])
                nc.gpsimd.dma_start(srep[:], sinr_d[:])
            for c in range(ch * 4, ch * 4 + 4):
                dmae[(c + 1) % 2].dma_start(out=xnT[c][:],
                                            in_=xnT_d[c * 128:(c + 1) * 128, :])
        nc.sync.dma_start(tri[:], tri_d[:])
        nc.sync.dma_start(perm[:], perm_d[:])
        nc.gpsimd.dma_start(wv_sb[:], wv_d.rearrange("(c p) h -> p c h", p=128))
        nc.sync.dma_start(wout_sb[:], wout_d.rearrange("(g p) e -> p g e", p=128))
        nc.vector.memset(onesb[:], 1.0)
        for t in range(n_tt):
            nc.vector.memset(vst[t][:, :, DH:DH + 1], 1.0)

        def rotary(ps_ap, dsl, tcl):
            """psum q/k [128, IB] -> dsl (bf16 slice), rotary applied.
            rotate_half's partition permutation runs on the PE (perm matmul);
            the sigma'd copy is consumed straight from PSUM."""
            raw = rot.tile([128, IB], BF16, tag="raw", name="raw")
            nc.vector.tensor_copy(raw[:], ps_ap[:])
            sq = ps.tile([128, IB], F32, tag="qps", name="sq")
            nc.tensor.matmul(sq[:], lhsT=perm[:], rhs=raw[:],
                             start=True, stop=True)
            tmp = rot.tile([128, IB], BF16, tag="tmp", name="tmp")
            nc.vector.tensor_mul(out=tmp[:], in0=raw[:], in1=crep[:, tcl])
            nc.vector.tensor_mul(out=dsl, in0=sq[:], in1=srep[:, tcl])
            nc.vector.tensor_add(out=dsl, in0=dsl, in1=tmp[:])

        def gen_kproj(hb, tc4):
            st = {}
            tcl = slice(tc4 * IB, (tc4 + 1) * IB)

            def mm(c0, c1):
                if c0 == 0:
                    st["ps"] = ps.tile([128, IB], F32, tag="qps", name="psk")
                for c in range(c0, c1):
                    nc.tensor.matmul(st["ps"][:],
                                     lhsT=wk_sb[:, c, hb * 128:(hb + 1) * 128],
                                     rhs=xnT[c][:, tcl],
                                     start=(c == 0), stop=(c == n_ct - 1))

            def fin():
                rotary(st["ps"], kt[hb][:, tcl], tcl)

            return [lambda: mm(0, 8), lambda: mm(8, 16), fin]

        def gen_qproj(ib, hb):
            st = {}
            tcl = slice(ib * IB, (ib + 1) * IB)

            def dma():
                slab = wqp.tile([128, n_ct, 128], BF16, tag="wq", name="wqs")
                dmae[(ib + hb) % 2].dma_start(
                    slab[:],
                    wq_d[:, hb * 128:(hb + 1) * 128].rearrange("(c p) m -> p c m",
                                                               p=128))
                st["slab"] = slab

            def mm(c0, c1):
                if c0 == 0:
                    st["ps"] = ps.tile([128, IB], F32, tag="qps", name="psq")
                for c in range(c0, c1):
                    nc.tensor.matmul(st["ps"][:], lhsT=st["slab"][:, c, :],
                                     rhs=xnT[c][:, tcl],
                                     start=(c == 0), stop=(c == n_ct - 1))

            def fin():
                qt_t = qtp.tile([128, IB], BF16, tag=f"qt{hb}", name=f"qt{hb}")
                qt_slot[(ib, hb)] = qt_t
                rotary(st["ps"], qt_t[:], tcl)

            return [dma, lambda: mm(0, 8), lambda: mm(8, 16), fin]

        def gen_vproj(t):
            st = {}

            def mm(c0, c1):
                if c0 == 0:
                    st["ps"] = ps.tile([128, IB], F32, tag="qps", name="psv")
                for c in range(c0, c1):
                    nc.tensor.matmul(st["ps"][:],
                                     lhsT=xnT[c][:, t * 128:(t + 1) * 128],
                                     rhs=wv_sb[:, c, :],
                                     start=(c == 0), stop=(c == n_ct - 1))

            def fin():
                nc.vector.tensor_copy(
                    vst[t][:, :, 0:DH],
                    st["ps"][:].rearrange("p (h d) -> p h d", h=HEADS))

            return [lambda: mm(0, 8), lambda: mm(8, 16), fin]

        qt_slot = {}
        ont_ap = {}

        def gen_outproj(i):
            """Wout projection of i-block i; bf16 [128, 2048] rows, 1 DMA per
            token block."""
            thunks = []
            for ic in range(4):
                st = {}
                for ec in range(4):
                    def th(ic=ic, ec=ec, st=st):
                        if ec == 0:
                            st["oc"] = ocp.tile([128, DIM], BF16, tag="oc",
                                                name="oc")
                        pso = ps.tile([128, 512], F32, tag="po", name="pso")
                        for hp in range(n_hb):
                            nc.tensor.matmul(
                                pso[:],
                                lhsT=ont_ap[(i, hp)][:, ic * 128:(ic + 1) * 128],
                                rhs=wout_sb[:, hp, ec * 512:(ec + 1) * 512],
                                start=(hp == 0), stop=(hp == n_hb - 1))
                        nc.any.tensor_copy(
                            st["oc"][:, ec * 512:(ec + 1) * 512], pso[:])
                        if ec == 3:
                            r0 = i * IB + ic * 128
                            dmae[(i + ic) % 2].dma_start(
                                out=out_d[r0:r0 + 128, :], in_=st["oc"][:])
                    thunks.append(th)
            return thunks

        def pipeline_units(units):
            """Flatten unit thunk-lists, delaying each unit's fin until after
            the next unit's first matmul thunk (so the fin's PE perm-matmul
            never head-of-line blocks the tensor queue on the DVE drain)."""
            seq = []
            pending = None
            for u in units:
                *body, fin = u
                seq.append(body[0])
                if pending is not None:
                    seq.append(pending)
                seq += body[1:]
                pending = fin
            if pending is not None:
                seq.append(pending)
            return seq

        # ---- phase 1: K (all), V (first 4 blocks), Q (i-block 0) ----
        units = [gen_kproj(hb, tc4) for hb in range(n_hb)
                 for tc4 in range(n_ib)]
        units += [gen_vproj(t) for t in range(jpi)]
        units += [gen_qproj(0, hb) for hb in range(n_hb)]
        for th in pipeline_units(units):
            th()

        # ---- phase 2: attention with pipelined filler ----
        pending = [None]
        for i in range(n_ib):
            funits = []
            if i < n_ib - 1:
                for hb in range(n_hb):
                    funits.append(gen_qproj(i + 1, hb))
                for t in range(jpi * (i + 1), jpi * (i + 2)):
                    funits.append(gen_vproj(t))
            fillers = pipeline_units(funits) if funits else []
            if i >= 2:
                fillers += gen_outproj(i - 2)
            if i == n_ib - 1:
                fillers += gen_outproj(i - 1)

            n_jb = jpi * (i + 1)
            total_steps = n_hb * n_jb
            done = 0
            step = 0
            for hp in range(n_hb):
                if pending[0] is not None and i == 0 and hp == 0:
                    pass  # flushed at jb==1 below
                o_ps = [ps_av.tile([DH + 1, IB], F32, tag=f"oav{k}",
                                   name=f"oav{k}") for k in (0, 1)]
                for jb in range(n_jb):
                    delta = jb - jpi * i
                    v0 = max(delta, 0) * 128
                    psl = slice(v0, IB)
                    jsl = slice(jb * 128, (jb + 1) * 128)
                    e_t = []
                    for k in (0, 1):
                        hsl = slice(k * 64, (k + 1) * 64)
                        s_ps = ps_sc.tile([128, IB], F32, tag=f"sc{k}",
                                          name=f"sc{k}")
                        nc.tensor.matmul(s_ps[:, psl], lhsT=kt[hp][hsl, jsl],
                                         rhs=qt_slot[(i, hp)][hsl, psl],
                                         start=True, stop=True)
                        e = epool.tile([128, IB], BF16, tag=f"e{k}",
                                       name=f"e{k}")
                        nc.scalar.activation(out=e[:, psl], in_=s_ps[:, psl],
                                             func=AF.Exp, scale=scale)
                        if delta >= 0:
                            dsl = slice(v0, v0 + 128)
                            nc.vector.tensor_mul(out=e[:, dsl], in0=e[:, dsl],
                                                 in1=tri[:])
                        e_t.append(e)
                    if jb == 1 and pending[0] is not None:
                        pending[0]()
                        pending[0] = None
                    # filler between scores and AV: absorbs the exp latency
                    step += 1
                    want = len(fillers) * step // total_steps
                    while done < want:
                        fillers[done]()
                        done += 1
                    for k in (0, 1):
                        h = hp * 2 + k
                        nc.tensor.matmul(o_ps[k][:, psl],
                                         lhsT=vst[jb][:, h, :],
                                         rhs=e_t[k][:, psl],
                                         start=(jb == 0), stop=(jb == n_jb - 1))

                # AV psum -> one sbuf bf16 tile (k0 rows 0:64, k1 rows
                # 64:128 via the psum-source partition shift); 1/d via the
                # single-instruction DVE approx reciprocal straight from the
                # psum denominator rows; rank-1 f32r broadcast; normalize.
                osb = osbp.tile([128, IB], BF16, tag="osb", name="osb")
                nc.vector.tensor_copy(osb[0:DH, :], o_ps[0][0:DH, :])
                nc.vector.tensor_copy(osb[64:128, :], o_ps[1][0:DH, :])
                rec = bcp.tile([128, 2 * IB], F32, tag="rec", name="rec")
                rec_bf = bcp.tile([128, 2 * IB], BF16, tag="recb", name="rec_bf")
                nc.vector.tensor_copy(rec[64:65, 0:IB],
                                      o_ps[0][DH:DH + 1, :])
                nc.vector.tensor_copy(rec[64:65, IB:2 * IB],
                                      o_ps[1][DH:DH + 1, :])
                with nc.allow_low_precision(reason="softmax 1/d bf16 ok"):
                    nc.vector.reciprocal(out=rec_bf[64:65, 0:IB],
                                         in_=rec[64:65, 0:IB])
                    nc.vector.reciprocal(out=rec_bf[64:65, IB:2 * IB],
                                         in_=rec[64:65, IB:2 * IB])

                def norm(i=i, hp=hp, osb=osb, rec_bf=rec_bf):
                    # deferred into the next head-pair's j-loop so the pbc
                    # matmuls never head-of-line block on the DVE chain
                    ont_t = ontp.tile([128, IB], BF16, tag=f"ont{hp}",
                                      name=f"ont{hp}")
                    ont_ap[(i, hp)] = ont_t
                    for k in (0, 1):
                        pbc = ps.tile([128, IB], F32, tag="po", name="pbc")
                        nc.tensor.matmul(
                            pbc[:], lhsT=onesb[64:65, 0:128],
                            rhs=rec_bf[64:65, k * IB:(k + 1) * IB],
                            start=True, stop=True)
                        nc.vector.tensor_mul(
                            out=ont_t[k * 64:(k + 1) * 64, :],
                            in0=pbc[k * 64:(k + 1) * 64, :],
                            in1=osb[k * 64:(k + 1) * 64, :])
                pending[0] = norm

            while done < len(fillers):
                fillers[done]()
                done += 1

        if pending[0] is not None:
            pending[0]()
            pending[0] = None
        for th in gen_outproj(n_ib - 1):
            th()

    nc.compile()
    return nc


def get_nc():
    if "nc" not in _CACHED:
        _CACHED["nc"] = _build()
    return _CACHED["nc"]


def host_inputs(x, rotary_emb, gamma, Wq, Wkv, Wout):
    """Build the 8 per-core input dicts."""
    bf = ml_dtypes.bfloat16
    x = np.asarray(x, np.float32)
    g = np.asarray(gamma, np.float32)
    nrm = np.sqrt((x * x).sum(-1, keepdims=True))
    xn = x / np.maximum(nrm, 1e-12) * (DIM ** 0.5) * g
    Wq = np.asarray(Wq, np.float32)
    Wkv = np.asarray(Wkv, np.float32)
    Wk = Wkv[:, :HEADS_TOTAL * DH]
    Wv = Wkv[:, HEADS_TOTAL * DH:]
    Wout = np.asarray(Wout, np.float32)
    pos = np.asarray(rotary_emb, np.float32)
    cos = np.cos(pos).T
    sgn = np.concatenate([-np.ones(DH // 2), np.ones(DH // 2)]).astype(np.float32)
    sin = (np.sin(pos) * sgn[None, :]).T
    cosr = np.ascontiguousarray(np.tile(cos, (2, 1)).astype(bf))
    sinr = np.ascontiguousarray(np.tile(sin, (2, 1)).astype(bf))
    jj, ii = np.mgrid[0:128, 0:128]
    tri = np.ascontiguousarray((jj <= ii).astype(bf))
    # rotate_half partition permutation: out row p reads row sigma(p); sigma
    # swaps the 32-halves within each 64-row head slot.
    sigma = (np.arange(128) % 64 // 32 * -64 + 32) + np.arange(128)
    perm = np.zeros((128, 128), np.float32)
    perm[sigma, np.arange(128)] = 1.0
    perm = np.ascontiguousarray(perm.astype(bf))
    maps = []
    for core in range(N_CORES):
        b, gq = core // GROUPS, core % GROUPS
        hs = slice(gq * HD, (gq + 1) * HD)
        maps.append({
            "xnT": np.ascontiguousarray(xn[b].T.astype(bf)),
            "wq": np.ascontiguousarray(Wq[:, hs].astype(bf)),
            "wk": np.ascontiguousarray(Wk[:, hs].astype(bf)),
            "wv": np.ascontiguousarray(Wv[:, hs].astype(bf)),
            "wout": np.ascontiguousarray(Wout[hs, :].astype(bf)),
            "cosr": cosr, "sinr": sinr, "tri": tri, "perm": perm,
        })
    return maps


def run_cores(in_maps, trace=False, **kwargs):
    from concourse.bass_utils import run_bass_kernel_spmd
    nc = get_nc()
    return run_bass_kernel_spmd(nc, in_maps, list(range(N_CORES)), trace=trace,
                                **kwargs)


def kernel(x, rotary_emb, gamma, Wq, Wkv, Wout):
    in_maps = host_inputs(x, rotary_emb, gamma, Wq, Wkv, Wout)
    res = run_cores(in_maps, trace=False)
    out = np.zeros((B, N, DIM), np.float32)
    for core in range(N_CORES):
        b = core // GROUPS
        out[b] += res.results[core]["out"].astype(np.float32)
    return out
